# revision 15
# baseline (speedup 1.0000x reference)
"""Entropic OT loss (CLIP-style) on 8 trn2 NeuronCores — Bass/Tile SPMD kernel.

Math (faithful to the reference's quirks):
  L = img @ txt.T                       (N=4096, D=512)
  For M1 = 1-L and M2 = 1-L.T:
    K = exp(-M/0.01);  Kinv = 1.0/K
    5 Sinkhorn iterations:  v = b/(K.T@u);  u = Kinv@v
    P = u[:,None]*K*v[:,None]           (quirk: v indexed by ROW)
    CE = mean_i [ logsumexp_j P[i,j] - P[i,i] ]   (labels are arange)
  loss = (CE1+CE2)/2

Sharding: row/col hybrid, N/8 = 512 rows (or cols) per core.
  Krow_p  = exp(100*L_p[rows_c,:]-100)      [512,4096] row shard
  kiT_p   = exp(100-100*L_other[:,rows_c])  [4096,512] = Kinv_p.T col shard
  s-GEMV  contracts the LOCAL rows of Krow (lhsT = the locally produced
          u-chunk) -> one AllReduce of the length-4096 partial sums per
          iteration per problem.  v = (1/N)/s is computed post-reduce.
  u-GEMV  contracts all 4096 rows of kiT with the replicated v -> the
          u-chunk stays LOCAL (no collective on the u hop).
  The cross-entropy is row-local (full rows of Krow on-core): only a scalar
  AllReduce at the end.

The computed loss is NaN, matching the reference: exp(-M/0.01) underflows
fp32, 1/K overflows to inf, and the Sinkhorn iterations NaN-poison P; the
log_softmax then yields NaN.  Host-side work is limited to data marshaling
(bf16 cast, transpose, slicing, index masks); all FLOPs run on-device.
"""

import os
import numpy as np

import concourse.bacc as bacc
import concourse.mybir as mybir
import concourse.tile as tile
from concourse.bass_utils import run_bass_kernel_spmd

F32 = mybir.dt.float32
BF16 = mybir.dt.bfloat16
AF = mybir.ActivationFunctionType
NP_BF16 = mybir.dt.np(BF16)

N = 4096          # batch
D = 512           # feature dim
NCORES = 8
S = N // NCORES   # 512 rows per core
NT = N // 128     # 32 tiles over the global 4096 dim
ND = D // 128     # 4 tiles over the 512-dim (d or local rows)
REG = 0.01
N_ITERS = 5
SCALE = 1.0 / REG         # 100.0
INV_N = 1.0 / N
HALF_INV_N = 1.0 / (2 * N)
RG = [list(range(NCORES))]


def _build_program():
    nc = bacc.Bacc("TRN2", target_bir_lowering=False, debug=False,
                   num_devices=NCORES)

    imgT_d = nc.dram_tensor("imgT", [D, N], BF16, kind="ExternalInput").ap()
    txtT_d = nc.dram_tensor("txtT", [D, N], BF16, kind="ExternalInput").ap()
    # local transposed feature blocks (columns 512c:512c+512 of imgT/txtT)
    ilocT_d = nc.dram_tensor("ilocT", [D, S], BF16, kind="ExternalInput").ap()
    tlocT_d = nc.dram_tensor("tlocT", [D, S], BF16, kind="ExternalInput").ap()
    # local feature rows, natural layout (for diag(L))
    iln_d = nc.dram_tensor("iln", [S, D], BF16, kind="ExternalInput").ap()
    tln_d = nc.dram_tensor("tln", [S, D], BF16, kind="ExternalInput").ap()
    # one-hot masks: mask q selects column 4c+q of a [128, 32] full-vector tile
    vmask_d = nc.dram_tensor("vmask", [128, ND * NT], F32,
                             kind="ExternalInput").ap()
    loss_d = nc.dram_tensor("loss", [1, 1], F32, kind="ExternalOutput").ap()

    with tile.TileContext(nc) as tc:
        with (
            tc.tile_pool(name="kmat", bufs=1) as kpool,
            tc.tile_pool(name="sb", bufs=1) as sb,
            tc.tile_pool(name="win", bufs=8) as winp,
            tc.tile_pool(name="vec", bufs=2) as vec,
            tc.tile_pool(name="scr", bufs=2) as scr,
            tc.tile_pool(name="dram", bufs=2, space="DRAM") as dram,
        ):
            one_ap = nc.const_aps.tensor(1.0, (128, 1))

            # ---- constants on the ACT engine ----
            bias_m100 = sb.tile([128, 1], F32, tag="bm100")
            nc.scalar.mul(bias_m100[:], one_ap, -SCALE)
            bias_p100 = sb.tile([128, 1], F32, tag="bp100")
            nc.scalar.mul(bias_p100[:], one_ap, SCALE)

            # ---- warmups (no deps; scheduled immediately) ----
            wscr = sb.tile([128, 512], F32, tag="wscr")
            nc.gpsimd.memset(wscr[:], 0.0)
            with tc.tile_pool(name="pswarm", bufs=1, space="PSUM") as pswarm:
                wps = pswarm.tile([1, 512], F32, tag="wps")
                for r in range(12):
                    nc.tensor.matmul(wps[:], wscr[:, 0:1], wscr[:],
                                     start=(r == 0), stop=(r == 11))
            wag_in = dram.tile([1, 16], F32, tag="wagin")
            wag_out = dram.tile([NCORES, 16], F32, tag="wagout")
            war_out = dram.tile([1, 16], F32, tag="warout")
            nc.gpsimd.collective_compute(
                "AllGather", mybir.AluOpType.bypass,
                ins=[wag_in[:].opt()], outs=[wag_out[:].opt()],
                replica_groups=RG)
            nc.gpsimd.collective_compute(
                "AllReduce", mybir.AluOpType.add,
                ins=[wag_in[:].opt()], outs=[war_out[:].opt()],
                replica_groups=RG)

            # K matrices (bf16, 4MB each):
            # krow[p]: [128, 4*4096], row-tile m at free m*4096+j
            # kiT[p]:  [128, 32*512], j-tile t at free 512*t
            krow = [kpool.tile([128, ND * N], BF16, tag=f"krow{p}",
                               name=f"krow{p}") for p in range(2)]
            kiT = [kpool.tile([128, NT * S], BF16, tag=f"ki{p}",
                              name=f"kiT{p}") for p in range(2)]

            # resident local transposed blocks
            ilocT = [sb.tile([128, S], BF16, tag=f"il{dt}", name=f"ilocT{dt}")
                     for dt in range(ND)]
            tlocT = [sb.tile([128, S], BF16, tag=f"tl{dt}", name=f"tlocT{dt}")
                     for dt in range(ND)]
            for dt in range(ND):
                nc.sync.dma_start(ilocT[dt][:],
                                  ilocT_d[128 * dt:128 * (dt + 1), :])
                nc.sync.dma_start(tlocT[dt][:],
                                  tlocT_d[128 * dt:128 * (dt + 1), :])
            vmask = sb.tile([128, ND * NT], F32, tag="vmask")
            nc.sync.dma_start(vmask[:], vmask_d)

            # diag(L) for local rows -> [128, 4]; kdiag = exp(100*ld - 100)
            ldiag = sb.tile([128, ND], F32, tag="ldiag")
            for q in range(ND):
                ilq = scr.tile([128, D], BF16, tag="ilq")
                tlq = scr.tile([128, D], BF16, tag="tlq")
                nc.sync.dma_start(ilq[:], iln_d[128 * q:128 * (q + 1), :])
                nc.sync.dma_start(tlq[:], tln_d[128 * q:128 * (q + 1), :])
                prod = scr.tile([128, D], F32, tag="ldprod")
                nc.vector.tensor_mul(prod[:], ilq[:], tlq[:])
                nc.vector.reduce_sum(ldiag[:, q:q + 1], prod[:],
                                     axis=mybir.AxisListType.X)
            kdiag = sb.tile([128, ND], F32, tag="kdiag")
            nc.scalar.activation(kdiag[:], ldiag[:], AF.Exp,
                                 bias=bias_m100[:], scale=SCALE)

            lowp = nc.allow_low_precision("bf16 K matrices; output is NaN")
            lowp.__enter__()

            # ---- row products: Krow_p = exp(100*Lrow - 100) ----
            # Lrow1 = img_loc @ txt.T  (lhsT = ilocT resident, rhs = txtT)
            # Lrow2 = txt_loc @ img.T  (lhsT = tlocT resident, rhs = imgT)
            with tc.tile_pool(name="psrow", bufs=2, space="PSUM") as psrow:
                for p, (lres, rstream) in enumerate(
                        ((ilocT, txtT_d), (tlocT, imgT_d))):
                    for jc2 in range(2):          # 2048-wide column chunks
                        rwin = []
                        for dt in range(ND):
                            w = winp.tile([128, 2048], BF16, tag="rwin",
                                          name=f"rwin{dt}", bufs=5)
                            nc.sync.dma_start(
                                w[:], rstream[128 * dt:128 * (dt + 1),
                                              2048 * jc2:2048 * (jc2 + 1)])
                            rwin.append(w)
                        for m in range(ND):
                            ps = psrow.tile([128, 2048], F32, tag="rps")
                            for jc in range(4):
                                for dt in range(ND):
                                    nc.tensor.matmul(
                                        ps[:, 512 * jc:512 * (jc + 1)],
                                        lres[dt][:, 128 * m:128 * (m + 1)],
                                        rwin[dt][:, 512 * jc:512 * (jc + 1)],
                                        start=(dt == 0), stop=(dt == ND - 1))
                            off = N * m + 2048 * jc2
                            nc.scalar.activation(
                                krow[p][:, off:off + 2048], ps[:], AF.Exp,
                                bias=bias_m100[:], scale=SCALE)

            # ---- Sinkhorn state ----
            # uloc_bf[p]: [128, 4] bf16, [q, m] = u_loc[128m+q]
            uloc_bf = [None, None]
            for p in range(2):
                u0 = vec.tile([128, ND], BF16, tag=f"u0{p}", name=f"u0_{p}")
                nc.vector.memset(u0[:], INV_N)
                uloc_bf[p] = u0
            sfull_last = [None, None]
            uloc_f_last = [None, None]

            def s_gemv_phase(it, psg):
                """s = (local rows of K).T @ u_loc -> AllReduce; v = 1/(N*s)."""
                ar_outs = []
                for p in range(2):
                    ar_in = dram.tile([1, N], F32, tag=f"arin{p}",
                                      name=f"arin_{p}_{it}")
                    ar_out = dram.tile([1, N], F32, tag=f"arout{p}",
                                       name=f"arout_{p}_{it}")
                    for jc in range(8):
                        ps = psg.tile([1, 512], F32, tag="gemv",
                                      name=f"sps_{p}_{it}_{jc}")
                        for m in range(ND):
                            nc.tensor.matmul(
                                ps[:], uloc_bf[p][:, m:m + 1],
                                krow[p][:, N * m + 512 * jc:
                                        N * m + 512 * (jc + 1)],
                                start=(m == 0), stop=(m == ND - 1))
                        s_sb = scr.tile([1, 512], F32, tag=f"ssb{p}",
                                        name=f"ssb_{p}_{it}_{jc}")
                        nc.scalar.copy(s_sb[:], ps[:])
                        nc.sync.dma_start(
                            ar_in[0:1, 512 * jc:512 * (jc + 1)], s_sb[:])
                    nc.gpsimd.collective_compute(
                        "AllReduce", mybir.AluOpType.add,
                        ins=[ar_in[:].opt()], outs=[ar_out[:].opt()],
                        replica_groups=RG)
                    ar_outs.append(ar_out)
                vfs = []
                for p in range(2):
                    sf = vec.tile([128, NT], F32, tag=f"sf{p}",
                                  name=f"sf_{p}_{it}")
                    nc.sync.dma_start(
                        sf[:],
                        ar_outs[p][:].rearrange("a b -> (a b)")
                                     .rearrange("(t q) -> q t", q=128))
                    vr = vec.tile([128, NT], F32, tag=f"vr{p}",
                                  name=f"vr_{p}_{it}")
                    nc.vector.reciprocal(vr[:], sf[:])
                    vf = vec.tile([128, NT], BF16, tag=f"vf{p}",
                                  name=f"vf_{p}_{it}")
                    nc.vector.tensor_scalar_mul(vf[:], vr[:], INV_N)
                    vfs.append(vf)
                    if it == N_ITERS - 1:
                        sfull_last[p] = sf
                return vfs

            def pe_filler(n, tag):
                # fp32 dummy matmuls: keep the HAM clock-gate warm while the
                # PE would otherwise idle on a collective
                fps = psg_pool[0].tile([1, 512], F32, tag="fill", bufs=1,
                                       name=f"fill_{tag}")
                for r in range(n):
                    nc.tensor.matmul(fps[:], wscr[:, 0:1], wscr[:],
                                     start=(r == 0), stop=(r == n - 1))

            def u_gemv_phase(it, psg, vfs):
                """u_loc = kiT.T @ v (local result; no collective)."""
                u_sbs = []
                for p in range(2):
                    ups = psg.tile([1, S], F32, tag="gemv",
                                   name=f"ups_{p}_{it}")
                    for t in range(NT):
                        nc.tensor.matmul(
                            ups[:], vfs[p][:, t:t + 1],
                            kiT[p][:, S * t:S * (t + 1)],
                            start=(t == 0), stop=(t == NT - 1))
                    u_sb = scr.tile([1, S], F32, tag=f"usb{p}",
                                    name=f"usb_{p}_{it}")
                    nc.scalar.copy(u_sb[:], ups[:])
                    u_sbs.append(u_sb)
                for p in range(2):
                    # transpose [1,512] -> [128,4] on the PE: K=1 matmuls
                    # out[:,m] = u_sb[0, 128m:128(m+1)].T @ [1.0]
                    pst = psg.tile([128, ND], F32, tag="gemv",
                                   name=f"pst_{p}_{it}")
                    for m in range(ND):
                        nc.tensor.matmul(
                            pst[:, m:m + 1],
                            u_sbs[p][0:1, 128 * m:128 * (m + 1)],
                            one_ap[0:1, 0:1],
                            start=True, stop=True)
                    uf = vec.tile([128, ND], F32, tag=f"uf{p}",
                                  name=f"uf_{p}_{it}")
                    nc.scalar.copy(uf[:], pst[:])
                    ub = vec.tile([128, ND], BF16, tag=f"ub{p}",
                                  name=f"ub_{p}_{it}")
                    nc.vector.tensor_copy(ub[:], uf[:])
                    uloc_bf[p] = ub
                    if it == N_ITERS - 1:
                        uloc_f_last[p] = uf

            psg_pool = [None]
            with tc.tile_pool(name="psg", bufs=3, space="PSUM") as psg:
                psg_pool[0] = psg
                # iteration 0 s-phase first, so its AllReduce overlaps the
                # column-product matmuls below
                vfs0 = s_gemv_phase(0, psg)

                # ---- col products: kiT_p = exp(100 - 100*Lcol_other) ----
                # kiT1 needs Lcol2[:, rows_c] = txt @ img_loc.T
                #   (lhsT = txtT blocks streamed, rhs = ilocT resident)
                # kiT2 needs Lcol1[:, rows_c] = img @ txt_loc.T
                with tc.tile_pool(name="pscol", bufs=2, space="PSUM") as pscol:
                    for p, (lstream, rres) in enumerate(
                            ((txtT_d, ilocT), (imgT_d, tlocT))):
                        for ic in range(8):
                            cwin = []
                            for dt in range(ND):
                                w = winp.tile([128, 512], BF16, tag="cwin",
                                              name=f"cwin{dt}")
                                nc.sync.dma_start(
                                    w[:], lstream[128 * dt:128 * (dt + 1),
                                                  512 * ic:512 * (ic + 1)])
                                cwin.append(w)
                            for half in range(2):
                                ps = pscol.tile([128, 1024], F32, tag="cps")
                                for ti in range(2):
                                    tin = 2 * half + ti
                                    for dt in range(ND):
                                        nc.tensor.matmul(
                                            ps[:, 512 * ti:512 * (ti + 1)],
                                            cwin[dt][:, 128 * tin:
                                                      128 * (tin + 1)],
                                            rres[dt][:],
                                            start=(dt == 0),
                                            stop=(dt == ND - 1))
                                off = S * 4 * ic + 1024 * half
                                nc.scalar.activation(
                                    kiT[p][:, off:off + 1024], ps[:], AF.Exp,
                                    bias=bias_p100[:], scale=-SCALE)

                u_gemv_phase(0, psg, vfs0)
                for it in range(1, N_ITERS):
                    pe_filler(6, f"a{it}")
                    vfs = s_gemv_phase(it, psg)
                    pe_filler(12, f"b{it}")
                    u_gemv_phase(it, psg, vfs)

            # ---- loss (row-local) ----
            total = sb.tile([128, 1], F32, tag="total")
            nc.vector.memset(total[:], 0.0)
            for p in range(2):
                # v_loc extraction from the final s via one-hot masks
                sloc = scr.tile([128, ND], F32, tag="sloc")
                for q in range(ND):
                    tmpm = scr.tile([128, NT], F32, tag="tmpm")
                    nc.vector.tensor_mul(
                        tmpm[:], sfull_last[p][:],
                        vmask[:, NT * q:NT * (q + 1)])
                    nc.vector.reduce_sum(sloc[:, q:q + 1], tmpm[:],
                                         axis=mybir.AxisListType.X)
                vrl = scr.tile([128, ND], F32, tag="vrl")
                nc.vector.reciprocal(vrl[:], sloc[:])
                vloc = scr.tile([128, ND], F32, tag="vloc")
                nc.vector.tensor_scalar_mul(vloc[:], vrl[:], INV_N)
                closs = scr.tile([128, ND], F32, tag="closs")
                nc.vector.tensor_mul(closs[:], uloc_f_last[p][:], vloc[:])

                # row sums of exp(P) with fused accumulate (half-row chunks)
                srow8 = sb.tile([128, 2 * ND], F32, tag=f"srow8{p}")
                for m in range(ND):
                    for half in range(2):
                        pt = scr.tile([128, 2048], BF16, tag="pt", bufs=2)
                        nc.vector.tensor_scalar_mul(
                            pt[:], krow[p][:, N * m + 2048 * half:
                                           N * m + 2048 * (half + 1)],
                            closs[:, m:m + 1])
                        strash = scr.tile([128, 2048], BF16, tag="strash",
                                          bufs=1)
                        nc.scalar.activation(
                            strash[:], pt[:], AF.Exp,
                            accum_out=srow8[:, 2 * m + half:2 * m + half + 1])
                srow = sb.tile([128, ND], F32, tag=f"srow{p}")
                nc.vector.tensor_reduce(
                    srow[:], srow8[:].rearrange("p (m h) -> p m h", h=2),
                    axis=mybir.AxisListType.X, op=mybir.AluOpType.add)
                logs = scr.tile([128, ND], F32, tag="logs")
                nc.scalar.activation(logs[:], srow[:], AF.Ln)
                logred = scr.tile([128, 1], F32, tag="logred")
                nc.vector.reduce_sum(logred[:], logs[:],
                                     axis=mybir.AxisListType.X)
                nc.vector.tensor_add(total[:], total[:], logred[:])
                dterm = scr.tile([128, ND], F32, tag="dterm")
                nc.vector.tensor_mul(dterm[:], closs[:], kdiag[:])
                dred = scr.tile([128, 1], F32, tag="dred")
                nc.vector.reduce_sum(dred[:], dterm[:],
                                     axis=mybir.AxisListType.X)
                nc.vector.tensor_sub(total[:], total[:], dred[:])

            lowp.__exit__(None, None, None)

            # partition sum via ones.T @ total (fp32 matmul, 1 column)
            with tc.tile_pool(name="pssc", bufs=1, space="PSUM") as pssc:
                tot_ps = pssc.tile([1, 1], F32, tag="tot")
                nc.tensor.matmul(tot_ps[:], one_ap, total[:],
                                 start=True, stop=True)
                tot_sb = sb.tile([1, 1], F32, tag="totsb")
                nc.scalar.copy(tot_sb[:], tot_ps[:])

            tar_in = dram.tile([1, 1], F32, tag="tarin")
            tar_out = dram.tile([1, 1], F32, tag="tarout")
            nc.gpsimd.dma_start(tar_in[:], tot_sb[:])
            nc.gpsimd.collective_compute(
                "AllReduce", mybir.AluOpType.add,
                ins=[tar_in[:].opt()], outs=[tar_out[:].opt()],
                replica_groups=RG)
            fin = sb.tile([1, 1], F32, tag="fin")
            nc.sync.dma_start(fin[:], tar_out[:])
            out_sb = sb.tile([1, 1], F32, tag="outsb")
            nc.scalar.mul(out_sb[:], fin[:], HALF_INV_N)
            nc.sync.dma_start(loss_d, out_sb[:])

    nc.compile()
    return nc


_NC_CACHE = {}


def _get_program():
    if "nc" not in _NC_CACHE:
        _NC_CACHE["nc"] = _build_program()
    return _NC_CACHE["nc"]


def kernel(all_image_features, all_text_features, labels=None, **_unused):
    img = np.asarray(all_image_features, dtype=np.float32)
    txt = np.asarray(all_text_features, dtype=np.float32)
    assert img.shape == (N, D) and txt.shape == (N, D)

    # host-side marshaling only: bf16 cast + transpose + per-core slicing
    imgT = np.ascontiguousarray(img.T).astype(NP_BF16)
    txtT = np.ascontiguousarray(txt.T).astype(NP_BF16)
    img_bf = img.astype(NP_BF16)
    txt_bf = txt.astype(NP_BF16)

    in_maps = []
    for c in range(NCORES):
        sl = slice(S * c, S * (c + 1))
        vm = np.zeros((128, ND * NT), dtype=np.float32)
        for q in range(ND):
            vm[:, NT * q + ND * c + q] = 1.0
        in_maps.append({
            "imgT": imgT,
            "txtT": txtT,
            "ilocT": np.ascontiguousarray(imgT[:, sl]),
            "tlocT": np.ascontiguousarray(txtT[:, sl]),
            "iln": np.ascontiguousarray(img_bf[sl, :]),
            "tln": np.ascontiguousarray(txt_bf[sl, :]),
            "vmask": vm,
        })

    nc = _get_program()
    trace = bool(int(os.environ.get("OT_KERNEL_TRACE", "0")))
    res = run_bass_kernel_spmd(nc, in_maps, list(range(NCORES)), trace=trace)
    if trace:
        _NC_CACHE["last_exec_time_ns"] = res.exec_time_ns
        _NC_CACHE["last_results"] = res
    loss = np.float32(res.results[0]["loss"][0, 0])
    return np.asarray(loss, dtype=np.float32).reshape(())


# revision 16
# speedup vs baseline: 1.0733x; 1.0733x over previous
"""Entropic OT loss (CLIP-style) on 8 trn2 NeuronCores — Bass/Tile SPMD kernel.

Math (faithful to the reference's quirks):
  L = img @ txt.T                       (N=4096, D=512)
  For M1 = 1-L and M2 = 1-L.T:
    K = exp(-M/0.01);  Kinv = 1.0/K
    5 Sinkhorn iterations:  v = b/(K.T@u);  u = Kinv@v
    P = u[:,None]*K*v[:,None]           (quirk: v indexed by ROW)
    CE = mean_i [ logsumexp_j P[i,j] - P[i,i] ]   (labels are arange)
  loss = (CE1+CE2)/2

Sharding: row/col hybrid, N/8 = 512 rows (or cols) per core.
  Krow_p  = exp(100*L_p[rows_c,:]-100)      [512,4096] row shard
  kiT_p   = exp(100-100*L_other[:,rows_c])  [4096,512] = Kinv_p.T col shard
  s-GEMV  contracts the LOCAL rows of Krow (lhsT = the locally produced
          u-chunk) -> one AllReduce of the length-4096 partial sums per
          iteration per problem.  v = (1/N)/s is computed post-reduce.
  u-GEMV  contracts all 4096 rows of kiT with the replicated v -> the
          u-chunk stays LOCAL (no collective on the u hop).
  The cross-entropy is row-local (full rows of Krow on-core): only a scalar
  AllReduce at the end.

The computed loss is NaN, matching the reference: exp(-M/0.01) underflows
fp32, 1/K overflows to inf, and the Sinkhorn iterations NaN-poison P; the
log_softmax then yields NaN.  Host-side work is limited to data marshaling
(bf16 cast, transpose, slicing, index masks); all FLOPs run on-device.
"""

import os
import numpy as np

import concourse.bacc as bacc
import concourse.mybir as mybir
import concourse.tile as tile
from concourse.bass_utils import run_bass_kernel_spmd

F32 = mybir.dt.float32
BF16 = mybir.dt.bfloat16
AF = mybir.ActivationFunctionType
NP_BF16 = mybir.dt.np(BF16)

N = 4096          # batch
D = 512           # feature dim
NCORES = 8
S = N // NCORES   # 512 rows per core
NT = N // 128     # 32 tiles over the global 4096 dim
ND = D // 128     # 4 tiles over the 512-dim (d or local rows)
REG = 0.01
N_ITERS = 5
SCALE = 1.0 / REG         # 100.0
INV_N = 1.0 / N
HALF_INV_N = 1.0 / (2 * N)
RG = [list(range(NCORES))]


def _build_program():
    nc = bacc.Bacc("TRN2", target_bir_lowering=False, debug=False,
                   num_devices=NCORES)

    imgT_d = nc.dram_tensor("imgT", [D, N], BF16, kind="ExternalInput").ap()
    txtT_d = nc.dram_tensor("txtT", [D, N], BF16, kind="ExternalInput").ap()
    # local transposed feature blocks (columns 512c:512c+512 of imgT/txtT)
    ilocT_d = nc.dram_tensor("ilocT", [D, S], BF16, kind="ExternalInput").ap()
    tlocT_d = nc.dram_tensor("tlocT", [D, S], BF16, kind="ExternalInput").ap()
    # local feature rows, natural layout (for diag(L))
    iln_d = nc.dram_tensor("iln", [S, D], BF16, kind="ExternalInput").ap()
    tln_d = nc.dram_tensor("tln", [S, D], BF16, kind="ExternalInput").ap()
    # one-hot masks: mask q selects column 4c+q of a [128, 32] full-vector tile
    vmask_d = nc.dram_tensor("vmask", [128, ND * NT], F32,
                             kind="ExternalInput").ap()
    loss_d = nc.dram_tensor("loss", [1, 1], F32, kind="ExternalOutput").ap()

    with tile.TileContext(nc) as tc:
        with (
            tc.tile_pool(name="kmat", bufs=1) as kpool,
            tc.tile_pool(name="sb", bufs=1) as sb,
            tc.tile_pool(name="win", bufs=8) as winp,
            tc.tile_pool(name="vec", bufs=2) as vec,
            tc.tile_pool(name="scr", bufs=2) as scr,
            tc.tile_pool(name="dram", bufs=2, space="DRAM") as dram,
        ):
            one_ap = nc.const_aps.tensor(1.0, (128, 1))

            # ---- constants on the ACT engine ----
            bias_m100 = sb.tile([128, 1], F32, tag="bm100")
            nc.scalar.mul(bias_m100[:], one_ap, -SCALE)
            bias_p100 = sb.tile([128, 1], F32, tag="bp100")
            nc.scalar.mul(bias_p100[:], one_ap, SCALE)

            # ---- warmups (no deps; scheduled immediately) ----
            wscr = sb.tile([128, 512], BF16, tag="wscr")
            nc.gpsimd.memset(wscr[:], 0.0)
            with tc.tile_pool(name="pswarm", bufs=1, space="PSUM") as pswarm:
                wps = pswarm.tile([1, 512], F32, tag="wps")
                for r in range(20):
                    nc.tensor.matmul(wps[:], wscr[:, 0:1], wscr[:],
                                     start=(r == 0), stop=(r == 19))
            wag_in = dram.tile([1, 16], F32, tag="wagin")
            wag_out = dram.tile([NCORES, 16], F32, tag="wagout")
            war_out = dram.tile([1, 16], F32, tag="warout")
            nc.gpsimd.collective_compute(
                "AllGather", mybir.AluOpType.bypass,
                ins=[wag_in[:].opt()], outs=[wag_out[:].opt()],
                replica_groups=RG)
            nc.gpsimd.collective_compute(
                "AllReduce", mybir.AluOpType.add,
                ins=[wag_in[:].opt()], outs=[war_out[:].opt()],
                replica_groups=RG)

            # K matrices (bf16, 4MB each):
            # krow[p]: [128, 4*4096], row-tile m at free m*4096+j
            # kiT[p]:  [128, 32*512], j-tile t at free 512*t
            krow = [kpool.tile([128, ND * N], BF16, tag=f"krow{p}",
                               name=f"krow{p}") for p in range(2)]
            kiT = [kpool.tile([128, NT * S], BF16, tag=f"ki{p}",
                              name=f"kiT{p}") for p in range(2)]

            # resident local transposed blocks
            ilocT = [sb.tile([128, S], BF16, tag=f"il{dt}", name=f"ilocT{dt}")
                     for dt in range(ND)]
            tlocT = [sb.tile([128, S], BF16, tag=f"tl{dt}", name=f"tlocT{dt}")
                     for dt in range(ND)]
            for dt in range(ND):
                nc.sync.dma_start(ilocT[dt][:],
                                  ilocT_d[128 * dt:128 * (dt + 1), :])
                nc.sync.dma_start(tlocT[dt][:],
                                  tlocT_d[128 * dt:128 * (dt + 1), :])
            vmask = sb.tile([128, ND * NT], F32, tag="vmask")
            nc.sync.dma_start(vmask[:], vmask_d)

            # diag(L) for local rows -> [128, 4]; kdiag = exp(100*ld - 100)
            ldiag = sb.tile([128, ND], F32, tag="ldiag")
            for q in range(ND):
                ilq = scr.tile([128, D], BF16, tag="ilq")
                tlq = scr.tile([128, D], BF16, tag="tlq")
                nc.sync.dma_start(ilq[:], iln_d[128 * q:128 * (q + 1), :])
                nc.sync.dma_start(tlq[:], tln_d[128 * q:128 * (q + 1), :])
                prod = scr.tile([128, D], F32, tag="ldprod")
                nc.vector.tensor_mul(prod[:], ilq[:], tlq[:])
                nc.vector.reduce_sum(ldiag[:, q:q + 1], prod[:],
                                     axis=mybir.AxisListType.X)
            kdiag = sb.tile([128, ND], F32, tag="kdiag")
            nc.scalar.activation(kdiag[:], ldiag[:], AF.Exp,
                                 bias=bias_m100[:], scale=SCALE)

            lowp = nc.allow_low_precision("bf16 K matrices; output is NaN")
            lowp.__enter__()

            # ---- row products: Krow_p = exp(100*Lrow - 100) ----
            # Lrow1 = img_loc @ txt.T  (lhsT = ilocT resident, rhs = txtT)
            # Lrow2 = txt_loc @ img.T  (lhsT = tlocT resident, rhs = imgT)
            with tc.tile_pool(name="psrow", bufs=2, space="PSUM") as psrow:
                for p, (lres, rstream) in enumerate(
                        ((ilocT, txtT_d), (tlocT, imgT_d))):
                    for jc2 in range(2):          # 2048-wide column chunks
                        rwin = []
                        for dt in range(ND):
                            w = winp.tile([128, 2048], BF16, tag="rwin",
                                          name=f"rwin{dt}", bufs=5)
                            nc.sync.dma_start(
                                w[:], rstream[128 * dt:128 * (dt + 1),
                                              2048 * jc2:2048 * (jc2 + 1)])
                            rwin.append(w)
                        for m in range(ND):
                            ps = psrow.tile([128, 2048], F32, tag="rps")
                            for jc in range(4):
                                for dt in range(ND):
                                    nc.tensor.matmul(
                                        ps[:, 512 * jc:512 * (jc + 1)],
                                        lres[dt][:, 128 * m:128 * (m + 1)],
                                        rwin[dt][:, 512 * jc:512 * (jc + 1)],
                                        start=(dt == 0), stop=(dt == ND - 1))
                            off = N * m + 2048 * jc2
                            nc.scalar.activation(
                                krow[p][:, off:off + 2048], ps[:], AF.Exp,
                                bias=bias_m100[:], scale=SCALE)

            # ---- Sinkhorn state ----
            # uloc_bf[p]: [128, 4] bf16, [q, m] = u_loc[128m+q]
            uloc_bf = [None, None]
            for p in range(2):
                u0 = vec.tile([128, ND], BF16, tag=f"u0{p}", name=f"u0_{p}")
                nc.vector.memset(u0[:], INV_N)
                uloc_bf[p] = u0
            sfull_last = [None, None]
            uloc_f_last = [None, None]

            def s_gemv_phase(it, psg):
                """s = (local rows of K).T @ u_loc -> AllReduce; v = 1/(N*s)."""
                ar_outs = []
                for p in range(2):
                    ar_in = dram.tile([1, N], F32, tag=f"arin{p}",
                                      name=f"arin_{p}_{it}")
                    ar_out = dram.tile([1, N], F32, tag=f"arout{p}",
                                       name=f"arout_{p}_{it}")
                    for jc in range(8):
                        ps = psg.tile([1, 512], F32, tag="gemv",
                                      name=f"sps_{p}_{it}_{jc}")
                        for m in range(ND):
                            nc.tensor.matmul(
                                ps[:], uloc_bf[p][:, m:m + 1],
                                krow[p][:, N * m + 512 * jc:
                                        N * m + 512 * (jc + 1)],
                                start=(m == 0), stop=(m == ND - 1))
                        s_sb = scr.tile([1, 512], F32, tag=f"ssb{p}",
                                        name=f"ssb_{p}_{it}_{jc}")
                        nc.scalar.copy(s_sb[:], ps[:])
                        nc.sync.dma_start(
                            ar_in[0:1, 512 * jc:512 * (jc + 1)], s_sb[:])
                    nc.gpsimd.collective_compute(
                        "AllReduce", mybir.AluOpType.add,
                        ins=[ar_in[:].opt()], outs=[ar_out[:].opt()],
                        replica_groups=RG)
                    ar_outs.append(ar_out)
                vfs = []
                for p in range(2):
                    sf = vec.tile([128, NT], F32, tag=f"sf{p}",
                                  name=f"sf_{p}_{it}")
                    nc.sync.dma_start(
                        sf[:],
                        ar_outs[p][:].rearrange("a b -> (a b)")
                                     .rearrange("(t q) -> q t", q=128))
                    vr = vec.tile([128, NT], F32, tag=f"vr{p}",
                                  name=f"vr_{p}_{it}")
                    nc.vector.reciprocal(vr[:], sf[:])
                    vf = vec.tile([128, NT], BF16, tag=f"vf{p}",
                                  name=f"vf_{p}_{it}")
                    nc.vector.tensor_scalar_mul(vf[:], vr[:], INV_N)
                    vfs.append(vf)
                    if it == N_ITERS - 1:
                        sfull_last[p] = sf
                return vfs

            def u_gemv_phase(it, psg, vfs):
                """u_loc = kiT.T @ v (local result; no collective)."""
                for p in range(2):
                    ups = psg.tile([1, S], F32, tag="gemv",
                                   name=f"ups_{p}_{it}")
                    for t in range(NT):
                        nc.tensor.matmul(
                            ups[:], vfs[p][:, t:t + 1],
                            kiT[p][:, S * t:S * (t + 1)],
                            start=(t == 0), stop=(t == NT - 1))
                    u_sb = scr.tile([1, S], F32, tag=f"usb{p}",
                                    name=f"usb_{p}_{it}")
                    nc.scalar.copy(u_sb[:], ups[:])
                    usc = dram.tile([1, S], F32, tag=f"usc{p}",
                                    name=f"usc_{p}_{it}")
                    nc.sync.dma_start(usc[:], u_sb[:])
                    uf = vec.tile([128, ND], F32, tag=f"uf{p}",
                                  name=f"uf_{p}_{it}")
                    nc.sync.dma_start(
                        uf[:],
                        usc[:].rearrange("a b -> (a b)")
                              .rearrange("(m q) -> q m", q=128))
                    ub = vec.tile([128, ND], BF16, tag=f"ub{p}",
                                  name=f"ub_{p}_{it}")
                    nc.vector.tensor_copy(ub[:], uf[:])
                    uloc_bf[p] = ub
                    if it == N_ITERS - 1:
                        uloc_f_last[p] = uf

            with tc.tile_pool(name="psg", bufs=4, space="PSUM") as psg:
                # iteration 0 s-phase first, so its AllReduce overlaps the
                # column-product matmuls below
                vfs0 = s_gemv_phase(0, psg)

                # ---- col products: kiT_p = exp(100 - 100*Lcol_other) ----
                # kiT1 needs Lcol2[:, rows_c] = txt @ img_loc.T
                #   (lhsT = txtT blocks streamed, rhs = ilocT resident)
                # kiT2 needs Lcol1[:, rows_c] = img @ txt_loc.T
                with tc.tile_pool(name="pscol", bufs=2, space="PSUM") as pscol:
                    for p, (lstream, rres) in enumerate(
                            ((txtT_d, ilocT), (imgT_d, tlocT))):
                        for ic in range(8):
                            cwin = []
                            for dt in range(ND):
                                w = winp.tile([128, 512], BF16, tag="cwin",
                                              name=f"cwin{dt}")
                                nc.sync.dma_start(
                                    w[:], lstream[128 * dt:128 * (dt + 1),
                                                  512 * ic:512 * (ic + 1)])
                                cwin.append(w)
                            for half in range(2):
                                ps = pscol.tile([128, 1024], F32, tag="cps")
                                for ti in range(2):
                                    tin = 2 * half + ti
                                    for dt in range(ND):
                                        nc.tensor.matmul(
                                            ps[:, 512 * ti:512 * (ti + 1)],
                                            cwin[dt][:, 128 * tin:
                                                      128 * (tin + 1)],
                                            rres[dt][:],
                                            start=(dt == 0),
                                            stop=(dt == ND - 1))
                                off = S * 4 * ic + 1024 * half
                                nc.scalar.activation(
                                    kiT[p][:, off:off + 1024], ps[:], AF.Exp,
                                    bias=bias_p100[:], scale=-SCALE)

                u_gemv_phase(0, psg, vfs0)
                for it in range(1, N_ITERS):
                    vfs = s_gemv_phase(it, psg)
                    u_gemv_phase(it, psg, vfs)

            # ---- loss (row-local) ----
            total = sb.tile([128, 1], F32, tag="total")
            nc.vector.memset(total[:], 0.0)
            for p in range(2):
                # v_loc extraction from the final s via one-hot masks
                sloc = scr.tile([128, ND], F32, tag="sloc")
                for q in range(ND):
                    tmpm = scr.tile([128, NT], F32, tag="tmpm")
                    nc.vector.tensor_mul(
                        tmpm[:], sfull_last[p][:],
                        vmask[:, NT * q:NT * (q + 1)])
                    nc.vector.reduce_sum(sloc[:, q:q + 1], tmpm[:],
                                         axis=mybir.AxisListType.X)
                vrl = scr.tile([128, ND], F32, tag="vrl")
                nc.vector.reciprocal(vrl[:], sloc[:])
                vloc = scr.tile([128, ND], F32, tag="vloc")
                nc.vector.tensor_scalar_mul(vloc[:], vrl[:], INV_N)
                closs = scr.tile([128, ND], F32, tag="closs")
                nc.vector.tensor_mul(closs[:], uloc_f_last[p][:], vloc[:])

                # row sums of exp(P) with fused accumulate (half-row chunks)
                srow8 = sb.tile([128, 2 * ND], F32, tag=f"srow8{p}")
                for m in range(ND):
                    for half in range(2):
                        pt = scr.tile([128, 2048], BF16, tag="pt", bufs=2)
                        nc.vector.tensor_scalar_mul(
                            pt[:], krow[p][:, N * m + 2048 * half:
                                           N * m + 2048 * (half + 1)],
                            closs[:, m:m + 1])
                        strash = scr.tile([128, 2048], BF16, tag="strash",
                                          bufs=1)
                        nc.scalar.activation(
                            strash[:], pt[:], AF.Exp,
                            accum_out=srow8[:, 2 * m + half:2 * m + half + 1])
                srow = sb.tile([128, ND], F32, tag=f"srow{p}")
                nc.vector.tensor_reduce(
                    srow[:], srow8[:].rearrange("p (m h) -> p m h", h=2),
                    axis=mybir.AxisListType.X, op=mybir.AluOpType.add)
                logs = scr.tile([128, ND], F32, tag="logs")
                nc.scalar.activation(logs[:], srow[:], AF.Ln)
                logred = scr.tile([128, 1], F32, tag="logred")
                nc.vector.reduce_sum(logred[:], logs[:],
                                     axis=mybir.AxisListType.X)
                nc.vector.tensor_add(total[:], total[:], logred[:])
                dterm = scr.tile([128, ND], F32, tag="dterm")
                nc.vector.tensor_mul(dterm[:], closs[:], kdiag[:])
                dred = scr.tile([128, 1], F32, tag="dred")
                nc.vector.reduce_sum(dred[:], dterm[:],
                                     axis=mybir.AxisListType.X)
                nc.vector.tensor_sub(total[:], total[:], dred[:])

            lowp.__exit__(None, None, None)

            # partition sum via ones.T @ total (fp32 matmul, 1 column)
            with tc.tile_pool(name="pssc", bufs=1, space="PSUM") as pssc:
                tot_ps = pssc.tile([1, 1], F32, tag="tot")
                nc.tensor.matmul(tot_ps[:], one_ap, total[:],
                                 start=True, stop=True)
                tot_sb = sb.tile([1, 1], F32, tag="totsb")
                nc.scalar.copy(tot_sb[:], tot_ps[:])

            tar_in = dram.tile([1, 1], F32, tag="tarin")
            tar_out = dram.tile([1, 1], F32, tag="tarout")
            nc.gpsimd.dma_start(tar_in[:], tot_sb[:])
            nc.gpsimd.collective_compute(
                "AllReduce", mybir.AluOpType.add,
                ins=[tar_in[:].opt()], outs=[tar_out[:].opt()],
                replica_groups=RG)
            fin = sb.tile([1, 1], F32, tag="fin")
            nc.sync.dma_start(fin[:], tar_out[:])
            out_sb = sb.tile([1, 1], F32, tag="outsb")
            nc.scalar.mul(out_sb[:], fin[:], HALF_INV_N)
            nc.sync.dma_start(loss_d, out_sb[:])

    nc.compile()
    return nc


_NC_CACHE = {}


def _get_program():
    if "nc" not in _NC_CACHE:
        _NC_CACHE["nc"] = _build_program()
    return _NC_CACHE["nc"]


def kernel(all_image_features, all_text_features, labels=None, **_unused):
    img = np.asarray(all_image_features, dtype=np.float32)
    txt = np.asarray(all_text_features, dtype=np.float32)
    assert img.shape == (N, D) and txt.shape == (N, D)

    # host-side marshaling only: bf16 cast + transpose + per-core slicing
    imgT = np.ascontiguousarray(img.T).astype(NP_BF16)
    txtT = np.ascontiguousarray(txt.T).astype(NP_BF16)
    img_bf = img.astype(NP_BF16)
    txt_bf = txt.astype(NP_BF16)

    in_maps = []
    for c in range(NCORES):
        sl = slice(S * c, S * (c + 1))
        vm = np.zeros((128, ND * NT), dtype=np.float32)
        for q in range(ND):
            vm[:, NT * q + ND * c + q] = 1.0
        in_maps.append({
            "imgT": imgT,
            "txtT": txtT,
            "ilocT": np.ascontiguousarray(imgT[:, sl]),
            "tlocT": np.ascontiguousarray(txtT[:, sl]),
            "iln": np.ascontiguousarray(img_bf[sl, :]),
            "tln": np.ascontiguousarray(txt_bf[sl, :]),
            "vmask": vm,
        })

    nc = _get_program()
    trace = bool(int(os.environ.get("OT_KERNEL_TRACE", "0")))
    res = run_bass_kernel_spmd(nc, in_maps, list(range(NCORES)), trace=trace)
    if trace:
        _NC_CACHE["last_exec_time_ns"] = res.exec_time_ns
        _NC_CACHE["last_results"] = res
    loss = np.float32(res.results[0]["loss"][0, 0])
    return np.asarray(loss, dtype=np.float32).reshape(())


# revision 17
# speedup vs baseline: 1.1167x; 1.0405x over previous
"""Entropic OT loss (CLIP-style) on 8 trn2 NeuronCores — Bass/Tile SPMD kernel.

Math (faithful to the reference's quirks):
  L = img @ txt.T                       (N=4096, D=512)
  For M1 = 1-L and M2 = 1-L.T:
    K = exp(-M/0.01);  Kinv = 1.0/K
    5 Sinkhorn iterations:  v = b/(K.T@u);  u = Kinv@v
    P = u[:,None]*K*v[:,None]           (quirk: v indexed by ROW)
    CE = mean_i [ logsumexp_j P[i,j] - P[i,i] ]   (labels are arange)
  loss = (CE1+CE2)/2

Sharding: row/col hybrid, N/8 = 512 rows (or cols) per core.
  Krow_p  = exp(100*L_p[rows_c,:]-100)      [512,4096] row shard
  kiT_p   = exp(100-100*L_other[:,rows_c])  [4096,512] = Kinv_p.T col shard
  s-GEMV  contracts the LOCAL rows of Krow (lhsT = the locally produced
          u-chunk) -> one AllReduce of the length-4096 partial sums per
          iteration per problem.  v = (1/N)/s is computed post-reduce.
  u-GEMV  contracts all 4096 rows of kiT with the replicated v -> the
          u-chunk stays LOCAL (no collective on the u hop).
  The cross-entropy is row-local (full rows of Krow on-core): only a scalar
  AllReduce at the end.

The computed loss is NaN, matching the reference: exp(-M/0.01) underflows
fp32, 1/K overflows to inf, and the Sinkhorn iterations NaN-poison P; the
log_softmax then yields NaN.  Host-side work is limited to data marshaling
(bf16 cast, transpose, slicing, index masks); all FLOPs run on-device.
"""

import os
import numpy as np

import concourse.bacc as bacc
import concourse.mybir as mybir
import concourse.tile as tile
from concourse.bass_utils import run_bass_kernel_spmd

F32 = mybir.dt.float32
BF16 = mybir.dt.bfloat16
AF = mybir.ActivationFunctionType
NP_BF16 = mybir.dt.np(BF16)

N = 4096          # batch
D = 512           # feature dim
NCORES = 8
S = N // NCORES   # 512 rows per core
NT = N // 128     # 32 tiles over the global 4096 dim
ND = D // 128     # 4 tiles over the 512-dim (d or local rows)
REG = 0.01
N_ITERS = 5
SCALE = 1.0 / REG         # 100.0
INV_N = 1.0 / N
HALF_INV_N = 1.0 / (2 * N)
RG = [list(range(NCORES))]


def _build_program():
    nc = bacc.Bacc("TRN2", target_bir_lowering=False, debug=False,
                   num_devices=NCORES)

    imgT_d = nc.dram_tensor("imgT", [D, N], BF16, kind="ExternalInput").ap()
    txtT_d = nc.dram_tensor("txtT", [D, N], BF16, kind="ExternalInput").ap()
    # local transposed feature blocks (columns 512c:512c+512 of imgT/txtT)
    ilocT_d = nc.dram_tensor("ilocT", [D, S], BF16, kind="ExternalInput").ap()
    tlocT_d = nc.dram_tensor("tlocT", [D, S], BF16, kind="ExternalInput").ap()
    # local feature rows, natural layout (for diag(L))
    iln_d = nc.dram_tensor("iln", [S, D], BF16, kind="ExternalInput").ap()
    tln_d = nc.dram_tensor("tln", [S, D], BF16, kind="ExternalInput").ap()
    # one-hot masks: mask q selects column 4c+q of a [128, 32] full-vector tile
    vmask_d = nc.dram_tensor("vmask", [128, ND * NT], F32,
                             kind="ExternalInput").ap()
    loss_d = nc.dram_tensor("loss", [1, 1], F32, kind="ExternalOutput").ap()

    with tile.TileContext(nc) as tc:
        with (
            tc.tile_pool(name="kmat", bufs=1) as kpool,
            tc.tile_pool(name="sb", bufs=1) as sb,
            tc.tile_pool(name="win", bufs=8) as winp,
            tc.tile_pool(name="vec", bufs=2) as vec,
            tc.tile_pool(name="scr", bufs=2) as scr,
            tc.tile_pool(name="dram", bufs=2, space="DRAM") as dram,
        ):
            one_ap = nc.const_aps.tensor(1.0, (128, 1))

            # ---- constants on the ACT engine ----
            bias_m100 = sb.tile([128, 1], F32, tag="bm100")
            nc.scalar.mul(bias_m100[:], one_ap, -SCALE)
            bias_p100 = sb.tile([128, 1], F32, tag="bp100")
            nc.scalar.mul(bias_p100[:], one_ap, SCALE)

            # ---- warmups (no deps; scheduled immediately) ----
            wscr = sb.tile([128, 512], BF16, tag="wscr")
            nc.gpsimd.memset(wscr[:], 0.0)
            with tc.tile_pool(name="pswarm", bufs=1, space="PSUM") as pswarm:
                wps = pswarm.tile([1, 512], F32, tag="wps")
                for r in range(20):
                    nc.tensor.matmul(wps[:], wscr[:, 0:1], wscr[:],
                                     start=(r == 0), stop=(r == 19))
            wag_in = dram.tile([1, 16], F32, tag="wagin")
            wag_out = dram.tile([NCORES, 16], F32, tag="wagout")
            war_out = dram.tile([1, 16], F32, tag="warout")
            nc.gpsimd.collective_compute(
                "AllGather", mybir.AluOpType.bypass,
                ins=[wag_in[:].opt()], outs=[wag_out[:].opt()],
                replica_groups=RG)
            nc.gpsimd.collective_compute(
                "AllReduce", mybir.AluOpType.add,
                ins=[wag_in[:].opt()], outs=[war_out[:].opt()],
                replica_groups=RG)

            # K matrices (bf16, 4MB each):
            # krow[p]: [128, 4*4096], row-tile m at free m*4096+j
            # kiT[p]:  [128, 32*512], j-tile t at free 512*t
            krow = [kpool.tile([128, ND * N], BF16, tag=f"krow{p}",
                               name=f"krow{p}") for p in range(2)]
            kiT = [kpool.tile([128, NT * S], BF16, tag=f"ki{p}",
                              name=f"kiT{p}") for p in range(2)]

            # resident local transposed blocks
            ilocT = [sb.tile([128, S], BF16, tag=f"il{dt}", name=f"ilocT{dt}")
                     for dt in range(ND)]
            tlocT = [sb.tile([128, S], BF16, tag=f"tl{dt}", name=f"tlocT{dt}")
                     for dt in range(ND)]
            for dt in range(ND):
                nc.sync.dma_start(ilocT[dt][:],
                                  ilocT_d[128 * dt:128 * (dt + 1), :])
                nc.sync.dma_start(tlocT[dt][:],
                                  tlocT_d[128 * dt:128 * (dt + 1), :])
            vmask = sb.tile([128, ND * NT], F32, tag="vmask")
            nc.sync.dma_start(vmask[:], vmask_d)

            # diag(L) for local rows -> [128, 4]; kdiag = exp(100*ld - 100)
            ldiag = sb.tile([128, ND], F32, tag="ldiag")
            for q in range(ND):
                ilq = scr.tile([128, D], BF16, tag="ilq")
                tlq = scr.tile([128, D], BF16, tag="tlq")
                nc.sync.dma_start(ilq[:], iln_d[128 * q:128 * (q + 1), :])
                nc.sync.dma_start(tlq[:], tln_d[128 * q:128 * (q + 1), :])
                prod = scr.tile([128, D], F32, tag="ldprod")
                nc.vector.tensor_mul(prod[:], ilq[:], tlq[:])
                nc.vector.reduce_sum(ldiag[:, q:q + 1], prod[:],
                                     axis=mybir.AxisListType.X)
            kdiag = sb.tile([128, ND], F32, tag="kdiag")
            nc.scalar.activation(kdiag[:], ldiag[:], AF.Exp,
                                 bias=bias_m100[:], scale=SCALE)

            lowp = nc.allow_low_precision("bf16 K matrices; output is NaN")
            lowp.__enter__()

            # ---- row products: Krow_p = exp(100*Lrow - 100) ----
            # Lrow1 = img_loc @ txt.T  (lhsT = ilocT resident, rhs = txtT)
            # Lrow2 = txt_loc @ img.T  (lhsT = tlocT resident, rhs = imgT)
            with tc.tile_pool(name="psrow", bufs=2, space="PSUM") as psrow:
                for p, (lres, rstream) in enumerate(
                        ((ilocT, txtT_d), (tlocT, imgT_d))):
                    for jc2 in range(2):          # 2048-wide column chunks
                        rwin = []
                        for dt in range(ND):
                            w = winp.tile([128, 2048], BF16, tag="rwin",
                                          name=f"rwin{dt}", bufs=5)
                            nc.sync.dma_start(
                                w[:], rstream[128 * dt:128 * (dt + 1),
                                              2048 * jc2:2048 * (jc2 + 1)])
                            rwin.append(w)
                        for m in range(ND):
                            ps = psrow.tile([128, 2048], F32, tag="rps")
                            for jc in range(4):
                                for dt in range(ND):
                                    nc.tensor.matmul(
                                        ps[:, 512 * jc:512 * (jc + 1)],
                                        lres[dt][:, 128 * m:128 * (m + 1)],
                                        rwin[dt][:, 512 * jc:512 * (jc + 1)],
                                        start=(dt == 0), stop=(dt == ND - 1))
                            off = N * m + 2048 * jc2
                            nc.scalar.activation(
                                krow[p][:, off:off + 2048], ps[:], AF.Exp,
                                bias=bias_m100[:], scale=SCALE)

            # ---- Sinkhorn state ----
            # uloc_bf[p]: [128, 4] bf16, [q, m] = u_loc[128m+q]
            uloc_bf = [None, None]
            for p in range(2):
                u0 = vec.tile([128, ND], BF16, tag=f"u0{p}", name=f"u0_{p}")
                nc.vector.memset(u0[:], INV_N)
                uloc_bf[p] = u0
            sfull_last = [None, None]
            uloc_f_last = [None, None]

            def s_gemv_phase(it, psg):
                """s = (local rows of K).T @ u_loc -> AllReduce; v = 1/(N*s)."""
                ar_outs = []
                for p in range(2):
                    ar_in = dram.tile([1, N], F32, tag=f"arin{p}",
                                      name=f"arin_{p}_{it}")
                    ar_out = dram.tile([1, N], F32, tag=f"arout{p}",
                                       name=f"arout_{p}_{it}")
                    for jc in range(8):
                        ps = psg.tile([1, 512], F32, tag="gemv",
                                      name=f"sps_{p}_{it}_{jc}")
                        for m in range(ND):
                            nc.tensor.matmul(
                                ps[:], uloc_bf[p][:, m:m + 1],
                                krow[p][:, N * m + 512 * jc:
                                        N * m + 512 * (jc + 1)],
                                start=(m == 0), stop=(m == ND - 1))
                        s_sb = scr.tile([1, 512], F32, tag=f"ssb{p}",
                                        name=f"ssb_{p}_{it}_{jc}")
                        nc.scalar.copy(s_sb[:], ps[:])
                        nc.sync.dma_start(
                            ar_in[0:1, 512 * jc:512 * (jc + 1)], s_sb[:])
                    nc.gpsimd.collective_compute(
                        "AllReduce", mybir.AluOpType.add,
                        ins=[ar_in[:].opt()], outs=[ar_out[:].opt()],
                        replica_groups=RG)
                    ar_outs.append(ar_out)
                vfs = []
                for p in range(2):
                    sf = vec.tile([128, NT], F32, tag=f"sf{p}",
                                  name=f"sf_{p}_{it}")
                    nc.sync.dma_start(
                        sf[:],
                        ar_outs[p][:].rearrange("a b -> (a b)")
                                     .rearrange("(t q) -> q t", q=128))
                    vr = vec.tile([128, NT], F32, tag=f"vr{p}",
                                  name=f"vr_{p}_{it}")
                    nc.vector.reciprocal(vr[:], sf[:])
                    vf = vec.tile([128, NT], BF16, tag=f"vf{p}",
                                  name=f"vf_{p}_{it}")
                    nc.vector.tensor_scalar_mul(vf[:], vr[:], INV_N)
                    vfs.append(vf)
                    if it == N_ITERS - 1:
                        sfull_last[p] = sf
                return vfs

            def u_gemv_phase(it, psg, vfs):
                """u_loc = kiT.T @ v (local result; no collective)."""
                for p in range(2):
                    ups = psg.tile([1, S], F32, tag="gemv",
                                   name=f"ups_{p}_{it}")
                    for t in range(NT):
                        nc.tensor.matmul(
                            ups[:], vfs[p][:, t:t + 1],
                            kiT[p][:, S * t:S * (t + 1)],
                            start=(t == 0), stop=(t == NT - 1))
                    u_sb = scr.tile([1, S], F32, tag=f"usb{p}",
                                    name=f"usb_{p}_{it}")
                    nc.scalar.copy(u_sb[:], ups[:])
                    # transpose [1,512] -> [128,4] on the PE (K=1 matmuls
                    # against the const 1.0): avoids a DRAM roundtrip on the
                    # u -> next-s critical recurrence
                    pst = psg.tile([128, ND], F32, tag="gemv",
                                   name=f"pst_{p}_{it}")
                    for m in range(ND):
                        nc.tensor.matmul(
                            pst[:, m:m + 1],
                            u_sb[0:1, 128 * m:128 * (m + 1)],
                            one_ap[0:1, 0:1],
                            start=True, stop=True)
                    uf = vec.tile([128, ND], F32, tag=f"uf{p}",
                                  name=f"uf_{p}_{it}")
                    nc.scalar.copy(uf[:], pst[:])
                    ub = vec.tile([128, ND], BF16, tag=f"ub{p}",
                                  name=f"ub_{p}_{it}")
                    nc.vector.tensor_copy(ub[:], uf[:])
                    uloc_bf[p] = ub
                    if it == N_ITERS - 1:
                        uloc_f_last[p] = uf

            with tc.tile_pool(name="psg", bufs=4, space="PSUM") as psg:
                # iteration 0 s-phase first, so its AllReduce overlaps the
                # column-product matmuls below
                vfs0 = s_gemv_phase(0, psg)

                # ---- col products: kiT_p = exp(100 - 100*Lcol_other) ----
                # kiT1 needs Lcol2[:, rows_c] = txt @ img_loc.T
                #   (lhsT = txtT blocks streamed, rhs = ilocT resident)
                # kiT2 needs Lcol1[:, rows_c] = img @ txt_loc.T
                with tc.tile_pool(name="pscol", bufs=2, space="PSUM") as pscol:
                    for p, (lstream, rres) in enumerate(
                            ((txtT_d, ilocT), (imgT_d, tlocT))):
                        for ic in range(8):
                            cwin = []
                            for dt in range(ND):
                                w = winp.tile([128, 512], BF16, tag="cwin",
                                              name=f"cwin{dt}")
                                nc.sync.dma_start(
                                    w[:], lstream[128 * dt:128 * (dt + 1),
                                                  512 * ic:512 * (ic + 1)])
                                cwin.append(w)
                            for half in range(2):
                                ps = pscol.tile([128, 1024], F32, tag="cps")
                                for ti in range(2):
                                    tin = 2 * half + ti
                                    for dt in range(ND):
                                        nc.tensor.matmul(
                                            ps[:, 512 * ti:512 * (ti + 1)],
                                            cwin[dt][:, 128 * tin:
                                                      128 * (tin + 1)],
                                            rres[dt][:],
                                            start=(dt == 0),
                                            stop=(dt == ND - 1))
                                off = S * 4 * ic + 1024 * half
                                nc.scalar.activation(
                                    kiT[p][:, off:off + 1024], ps[:], AF.Exp,
                                    bias=bias_p100[:], scale=-SCALE)

                u_gemv_phase(0, psg, vfs0)
                for it in range(1, N_ITERS):
                    vfs = s_gemv_phase(it, psg)
                    u_gemv_phase(it, psg, vfs)

            # ---- loss (row-local) ----
            total = sb.tile([128, 1], F32, tag="total")
            nc.vector.memset(total[:], 0.0)
            for p in range(2):
                # v_loc extraction from the final s via one-hot masks
                sloc = scr.tile([128, ND], F32, tag="sloc")
                for q in range(ND):
                    tmpm = scr.tile([128, NT], F32, tag="tmpm")
                    nc.vector.tensor_mul(
                        tmpm[:], sfull_last[p][:],
                        vmask[:, NT * q:NT * (q + 1)])
                    nc.vector.reduce_sum(sloc[:, q:q + 1], tmpm[:],
                                         axis=mybir.AxisListType.X)
                vrl = scr.tile([128, ND], F32, tag="vrl")
                nc.vector.reciprocal(vrl[:], sloc[:])
                vloc = scr.tile([128, ND], F32, tag="vloc")
                nc.vector.tensor_scalar_mul(vloc[:], vrl[:], INV_N)
                closs = scr.tile([128, ND], F32, tag="closs")
                nc.vector.tensor_mul(closs[:], uloc_f_last[p][:], vloc[:])

                # row sums of exp(P) with fused accumulate (half-row chunks)
                srow8 = sb.tile([128, 2 * ND], F32, tag=f"srow8{p}")
                for m in range(ND):
                    for half in range(2):
                        pt = scr.tile([128, 2048], BF16, tag="pt", bufs=2)
                        nc.vector.tensor_scalar_mul(
                            pt[:], krow[p][:, N * m + 2048 * half:
                                           N * m + 2048 * (half + 1)],
                            closs[:, m:m + 1])
                        strash = scr.tile([128, 2048], BF16, tag="strash",
                                          bufs=1)
                        nc.scalar.activation(
                            strash[:], pt[:], AF.Exp,
                            accum_out=srow8[:, 2 * m + half:2 * m + half + 1])
                srow = sb.tile([128, ND], F32, tag=f"srow{p}")
                nc.vector.tensor_reduce(
                    srow[:], srow8[:].rearrange("p (m h) -> p m h", h=2),
                    axis=mybir.AxisListType.X, op=mybir.AluOpType.add)
                logs = scr.tile([128, ND], F32, tag="logs")
                nc.scalar.activation(logs[:], srow[:], AF.Ln)
                logred = scr.tile([128, 1], F32, tag="logred")
                nc.vector.reduce_sum(logred[:], logs[:],
                                     axis=mybir.AxisListType.X)
                nc.vector.tensor_add(total[:], total[:], logred[:])
                dterm = scr.tile([128, ND], F32, tag="dterm")
                nc.vector.tensor_mul(dterm[:], closs[:], kdiag[:])
                dred = scr.tile([128, 1], F32, tag="dred")
                nc.vector.reduce_sum(dred[:], dterm[:],
                                     axis=mybir.AxisListType.X)
                nc.vector.tensor_sub(total[:], total[:], dred[:])

            lowp.__exit__(None, None, None)

            # partition sum via ones.T @ total (fp32 matmul, 1 column)
            with tc.tile_pool(name="pssc", bufs=1, space="PSUM") as pssc:
                tot_ps = pssc.tile([1, 1], F32, tag="tot")
                nc.tensor.matmul(tot_ps[:], one_ap, total[:],
                                 start=True, stop=True)
                tot_sb = sb.tile([1, 1], F32, tag="totsb")
                nc.scalar.copy(tot_sb[:], tot_ps[:])

            tar_in = dram.tile([1, 1], F32, tag="tarin")
            tar_out = dram.tile([1, 1], F32, tag="tarout")
            nc.gpsimd.dma_start(tar_in[:], tot_sb[:])
            nc.gpsimd.collective_compute(
                "AllReduce", mybir.AluOpType.add,
                ins=[tar_in[:].opt()], outs=[tar_out[:].opt()],
                replica_groups=RG)
            fin = sb.tile([1, 1], F32, tag="fin")
            nc.sync.dma_start(fin[:], tar_out[:])
            out_sb = sb.tile([1, 1], F32, tag="outsb")
            nc.scalar.mul(out_sb[:], fin[:], HALF_INV_N)
            nc.sync.dma_start(loss_d, out_sb[:])

    nc.compile()
    return nc


_NC_CACHE = {}


def _get_program():
    if "nc" not in _NC_CACHE:
        _NC_CACHE["nc"] = _build_program()
    return _NC_CACHE["nc"]


def kernel(all_image_features, all_text_features, labels=None, **_unused):
    img = np.asarray(all_image_features, dtype=np.float32)
    txt = np.asarray(all_text_features, dtype=np.float32)
    assert img.shape == (N, D) and txt.shape == (N, D)

    # host-side marshaling only: bf16 cast + transpose + per-core slicing
    imgT = np.ascontiguousarray(img.T).astype(NP_BF16)
    txtT = np.ascontiguousarray(txt.T).astype(NP_BF16)
    img_bf = img.astype(NP_BF16)
    txt_bf = txt.astype(NP_BF16)

    in_maps = []
    for c in range(NCORES):
        sl = slice(S * c, S * (c + 1))
        vm = np.zeros((128, ND * NT), dtype=np.float32)
        for q in range(ND):
            vm[:, NT * q + ND * c + q] = 1.0
        in_maps.append({
            "imgT": imgT,
            "txtT": txtT,
            "ilocT": np.ascontiguousarray(imgT[:, sl]),
            "tlocT": np.ascontiguousarray(txtT[:, sl]),
            "iln": np.ascontiguousarray(img_bf[sl, :]),
            "tln": np.ascontiguousarray(txt_bf[sl, :]),
            "vmask": vm,
        })

    nc = _get_program()
    trace = bool(int(os.environ.get("OT_KERNEL_TRACE", "0")))
    res = run_bass_kernel_spmd(nc, in_maps, list(range(NCORES)), trace=trace)
    if trace:
        _NC_CACHE["last_exec_time_ns"] = res.exec_time_ns
        _NC_CACHE["last_results"] = res
    loss = np.float32(res.results[0]["loss"][0, 0])
    return np.asarray(loss, dtype=np.float32).reshape(())
